# revision 1
# baseline (speedup 1.0000x reference)
"""Multi-head attention (B=4, T=S=2048, E=1024, H=16) on 8 trn2 NeuronCores.

Sharding: core c handles batch b = c // 2 and head-half hh = c % 2
(8 of 16 heads).  Each core computes its heads' Q/K/V projections,
attention, and a partial output projection (contraction over its 512
e-dims).  The host sums the two partial outputs per batch and adds bo.

On-chip layout is "transposed activations": scores are computed as
scores.T = kp @ qp.T  ([s, t], s on partitions), softmax denominators
come from an extra ones-column in the PV matmul (M=65), and the output
projection consumes ctx.T directly, producing out.T per core.
Activations/weights are transposed on-chip via PE (tensor-engine)
transposes after a DVE fp32->bf16 cast.
"""

import numpy as np

import concourse.bass as bass
import concourse.mybir as mybir
import concourse.tile as tile
from concourse.bass_utils import run_bass_kernel_spmd
from concourse.masks import make_identity

F32 = mybir.dt.float32
BF16 = mybir.dt.bfloat16

B, T, E = 4, 2048, 1024
H = 16  # global heads
HL = 8  # heads per core (local)
HD = 64  # head dim
EL = HL * HD  # 512, e-dims per core
N_CORES = 8

_CACHED = {}


def legalize_waits(nc, cap=1):
    """Hoist semaphore waits so no instruction carries more than `cap`.

    The cayman 64B ISA instruction format has a single wait slot
    (NEURON_ISA_TPB_EVENTS); this container's walrus rejects instructions
    with more attached waits ("Too many sync wait commands").  Tile's sem
    assignment freely attaches several, so we split the excess onto
    standalone InstEventSemaphore carriers (exactly what raw-bass
    wait_ge emits) on the same engine, immediately before.
    """
    import bass_rust

    # Pass 1: statically-known final value of every semaphore (sum of all
    # attached increments) — needed to replace the tail RANGE_CLEAR (an
    # InstISA opcode this walrus can't codegen) with sem-dec updates.
    totals = {}
    names = {}
    for f in nc.m.functions:
        for bb in f.blocks:
            for ins in bb.instructions:
                si = ins.sync_info
                if si is None:
                    continue
                for u in si.on_update or []:
                    if u.sync_type == "semaphore":
                        sign = 1 if u.update_mode in ("sem-inc", "sem-add-imm") else -1
                        totals[u.id] = totals.get(u.id, 0) + sign * u.update_value
                        names[u.id] = u.ant_name

    n = 0
    for f in nc.m.functions:
        for bb in f.blocks:
            insts = bb.instructions
            out = []
            changed = False
            for ins in insts:
                if type(ins).__name__ == "InstISA" and "RANGE_CLEAR" in str(ins):
                    import re

                    m = re.search(r"range_first=(\d+) range_last=(\d+)", str(ins))
                    first, last = int(m.group(1)), int(m.group(2))
                    for sid in range(first, last + 1):
                        tot = totals.get(sid, 0)
                        if tot == 0:
                            continue
                        ev = mybir.InstEventSemaphore(name=f"I-LC{n}", ins=[], outs=[])
                        n += 1
                        ev.engine = ins.engine
                        ev.sync_info = bass_rust.SyncInfo(
                            on_wait=[],
                            on_update=[
                                bass_rust.SyncUpdate(
                                    sync_type="semaphore",
                                    id=sid,
                                    ant_name=names.get(sid, f"sem{sid}"),
                                    update_mode="sem-sub-imm",
                                    update_value=tot,
                                    update_reg=None,
                                )
                            ],
                        )
                        out.append(ev)
                    changed = True
                    continue
                si = ins.sync_info
                ws = list(si.on_wait) if (si is not None and si.on_wait) else []
                if len(ws) > cap:
                    for w in ws[: len(ws) - cap]:
                        ev = mybir.InstEventSemaphore(
                            name=f"I-LW{n}", ins=[], outs=[]
                        )
                        n += 1
                        ev.engine = ins.engine
                        ev.sync_info = bass_rust.SyncInfo(
                            on_wait=[w], on_update=[]
                        )
                        out.append(ev)
                    si.on_wait = ws[len(ws) - cap :]
                    changed = True
                out.append(ins)
            if changed:
                insts[:] = out
    return n


def build_program():
    nc = bass.Bass()

    qd = nc.declare_dram_parameter("q", [T, E], F32, isOutput=False)
    kd = nc.declare_dram_parameter("k", [T, E], F32, isOutput=False)
    vd = nc.declare_dram_parameter("v", [T, E], F32, isOutput=False)
    wqd = nc.declare_dram_parameter("wq", [EL, E], F32, isOutput=False)
    wkd = nc.declare_dram_parameter("wk", [EL, E], F32, isOutput=False)
    wvd = nc.declare_dram_parameter("wv", [EL, E], F32, isOutput=False)
    wod = nc.declare_dram_parameter("wo", [E, EL], F32, isOutput=False)
    bqd = nc.declare_dram_parameter("bq", [EL], F32, isOutput=False)
    bkd = nc.declare_dram_parameter("bk", [EL], F32, isOutput=False)
    bvd = nc.declare_dram_parameter("bv", [EL], F32, isOutput=False)
    outd = nc.declare_dram_parameter("outT", [E, T], F32, isOutput=True)

    with tile.TileContext(nc, pool_alloc_mode="queue") as tc:
        with (
            tc.tile_pool(name="singles", bufs=1) as singles,
            tc.tile_pool(name="stage", bufs=2) as stage,
            tc.tile_pool(name="xt", bufs=1) as xtp,
            tc.tile_pool(name="acts", bufs=1) as acts,
            tc.tile_pool(name="pt", bufs=6) as ptp,
            tc.tile_pool(name="norm", bufs=4) as normp,
            tc.tile_pool(name="osb", bufs=4) as osbp,
            tc.tile_pool(name="dram", bufs=2, space="DRAM") as dramp,
            tc.tile_pool(name="proj_ps", bufs=2, space="PSUM") as proj_ps,
            tc.tile_pool(name="sc_ps", bufs=2, space="PSUM") as sc_ps,
            tc.tile_pool(name="ctx_ps", bufs=2, space="PSUM") as ctx_ps,
        ):
            # ---------------- prologue: weights / biases / consts ----------
            ident = singles.tile([128, 128], BF16)
            make_identity(nc, ident)

            # Transposed bf16 weights:
            #   WqT[p, c, m] = Wq_c[m, c*128 + p]   (c,p) = e in [0,1024)
            wqT = singles.tile([128, 8, EL], BF16)
            wkT = singles.tile([128, 8, EL], BF16)
            wvT = singles.tile([128, 8, EL], BF16)
            # WoT[p, c, o] = Wo_c[o, c*128 + p]     (c,p) = local e in [0,512)
            woT = singles.tile([128, 4, E], BF16)

            def load_cast(xd, nrows):
                """Load [nrows, ncols] f32 DRAM -> bf16 SBUF natural tile
                [128, nrows//128, ncols] (cast happens in the SWDGE DMA).
                Chunked by 512 rows so downstream PE transposes can start
                before the whole tensor has landed."""
                a = nrows // 128
                ncols = xd.shape[1]
                xb = stage.tile([128, a, ncols], BF16, tag="stage_b")
                step = min(4, a)
                for blk in range(0, a, step):
                    nc.gpsimd.dma_start(
                        out=xb[:, blk : blk + step, :],
                        in_=xd[blk * 128 : (blk + step) * 128, :].rearrange(
                            "(a p) e -> p a e", p=128
                        ),
                    )
                return xb

            def pe_transpose(dst, src, a_chunks, e_chunks):
                """dst[p, e, a*128 + t] = src[t(p), a, e*128 + p] via PE
                transposes; dst is [128, e_chunks, a_chunks*128]."""
                for e in range(e_chunks):
                    # stack the a_chunks transposes of e-chunk e into one
                    # psum tile, then copy out in one DVE op
                    n = a_chunks * 128
                    tr = sc_ps.tile([128, n], BF16, tag="sc")
                    for a in range(a_chunks):
                        nc.tensor.transpose(
                            tr[:, a * 128 : (a + 1) * 128],
                            src[:, a, e * 128 : (e + 1) * 128],
                            ident,
                        )
                    nc.vector.tensor_copy(out=dst[:, e, :], in_=tr)

            # wv first: the v projection is the first consumer of any weight,
            # so its transpose should be ready earliest.
            for wd, wT in ((wvd, wvT), (wqd, wqT), (wkd, wkT)):
                wb = load_cast(wd, EL)
                pe_transpose(wT, wb, a_chunks=4, e_chunks=8)
            wob = load_cast(wod, E)
            pe_transpose(woT, wob, a_chunks=8, e_chunks=4)

            # biases: bq_sb[p, c] = bq[c*128 + p]
            bq_sb = singles.tile([128, 4], F32)
            bk_sb = singles.tile([128, 4], F32)
            nc.gpsimd.dma_start(out=bq_sb, in_=bqd.rearrange("(c p) -> p c", p=128))
            nc.gpsimd.dma_start(out=bk_sb, in_=bkd.rearrange("(c p) -> p c", p=128))
            bv_sb = singles.tile([1, EL], BF16)
            nc.gpsimd.dma_start(out=bv_sb, in_=bvd.rearrange("(o e) -> o e", o=1))
            ones_col = singles.tile([1, 128], BF16)
            nc.vector.memset(ones_col, 1.0)
            ones64b = singles.tile([1, 64], BF16)
            nc.vector.memset(ones64b, 1.0)

            # ---------------- projections --------------------------------
            # qpT[p, j, t] = qp[t, j*128 + p]  (pair j: head 2j at p<64)
            qpT = acts.tile([128, 4, T], BF16)
            kpT = acts.tile([128, 4, T], BF16)
            # vp_ext[p, s, h*65 + d] = vp[s*128 + p, h*64 + d]; col h*65+64 = 1.0
            vp_ext = acts.tile([128, 16, HL * 65], BF16)

            def load_xT(xd):
                """x [T, E] f32 DRAM -> xT[p, c, t] = x[t, c*128 + p] bf16."""
                xT = xtp.tile([128, 8, T], BF16, tag="xT")
                for blk in range(4):
                    xb = load_cast(xd[blk * 512 : (blk + 1) * 512, :], 512)
                    # xb[pt, a, e]: t = blk*512 + a*128 + pt
                    for e in range(8):
                        tr = sc_ps.tile([128, 512], BF16, tag="sc")
                        for a in range(4):
                            nc.tensor.transpose(
                                tr[:, a * 128 : (a + 1) * 128],
                                xb[:, a, e * 128 : (e + 1) * 128],
                                ident,
                            )
                        nc.vector.tensor_copy(
                            out=xT[:, e, blk * 512 : (blk + 1) * 512], in_=tr
                        )
                return xT

            # k and q first: scores (and the ACT exp stream) depend only on
            # kpT/qpT, while vp_ext is consumed per-s-chunk by PV later.
            for xd, xpT, b_sb, wT in ((kd, kpT, bk_sb, wkT), (qd, qpT, bq_sb, wqT)):
                xT = load_xT(xd)
                for c in range(4):
                    for tb in range(4):
                        ps = proj_ps.tile([128, 512], F32, tag="proj")
                        for e in range(8):
                            nc.tensor.matmul(
                                ps,
                                lhsT=wT[:, e, c * 128 : (c + 1) * 128],
                                rhs=xT[:, e, tb * 512 : (tb + 1) * 512],
                                start=(e == 0),
                                stop=(e == 7),
                            )
                        nc.vector.tensor_scalar_add(
                            out=xpT[:, c, tb * 512 : (tb + 1) * 512],
                            in0=ps,
                            scalar1=b_sb[:, c : c + 1],
                        )

            vT = load_xT(vd)
            for s in range(16):
                ps = proj_ps.tile([128, 512], F32, tag="proj")
                for e in range(8):
                    nc.tensor.matmul(
                        ps,
                        lhsT=vT[:, e, s * 128 : (s + 1) * 128],
                        rhs=wvT[:, e, :],
                        start=(e == 0),
                        stop=False,
                    )
                # += ones ⊗ bv  (bias along the free dim)
                nc.tensor.matmul(ps, lhsT=ones_col, rhs=bv_sb, start=False, stop=True)
                nc.vector.memset(vp_ext[:, s, :], 1.0)
                nc.vector.tensor_copy(
                    out=vp_ext[:, s, :].rearrange("p (h x) -> p h x", x=65)[:, :, 0:64],
                    in_=ps.rearrange("p (h d) -> p h d", d=64),
                )

            # ---------------- attention ----------------------------------
            # ctxn[p, j, t] = ctx[t, j*128 + p] / denom
            ctxn = acts.tile([128, 4, T], BF16)

            for tb in range(4):
                tsl = slice(tb * 512, (tb + 1) * 512)
                for j in range(4):
                    hA, hB = 2 * j, 2 * j + 1
                    ctx_a = ctx_ps.tile([65, 512], F32, tag="ctx")
                    ctx_b = ctx_ps.tile([65, 512], F32, tag="ctx")
                    for s in range(16):
                        ssl = slice(s * 128, (s + 1) * 128)
                        sc = sc_ps.tile([128, 1024], F32, tag="sc")
                        # scores.T tiles, row-packed pair (K=64 each)
                        nc.tensor.matmul(
                            sc[:, 0:512],
                            lhsT=kpT[0:64, j, ssl],
                            rhs=qpT[0:64, j, tsl],
                            start=True,
                            stop=True,
                        )
                        nc.tensor.matmul(
                            sc[:, 512:1024],
                            lhsT=kpT[64:128, j, ssl],
                            rhs=qpT[64:128, j, tsl],
                            start=True,
                            stop=True,
                        )
                        pt = ptp.tile([128, 1024], BF16, tag="pt")
                        nc.scalar.activation(
                            out=pt,
                            in_=sc,
                            func=mybir.ActivationFunctionType.Exp,
                            scale=0.125,
                        )
                        nc.tensor.matmul(
                            ctx_a,
                            lhsT=vp_ext[:, s, hA * 65 : hA * 65 + 65],
                            rhs=pt[:, 0:512],
                            start=(s == 0),
                            stop=(s == 15),
                        )
                        nc.tensor.matmul(
                            ctx_b,
                            lhsT=vp_ext[:, s, hB * 65 : hB * 65 + 65],
                            rhs=pt[:, 512:1024],
                            start=(s == 0),
                            stop=(s == 15),
                        )
                    # normalize: row 64 of ctx_* holds the denominators
                    # Drain ctx PSUM to SBUF right away so the next unit's PV
                    # can start; normalize off the critical path from SBUF.
                    ctxu = normp.tile([65, 1024], F32, tag="ctxu", bufs=2)
                    nc.vector.tensor_copy(out=ctxu[:, 0:512], in_=ctx_a)
                    nc.vector.tensor_copy(out=ctxu[:, 512:1024], in_=ctx_b)
                    # reciprocal of the denominators, then broadcast across
                    # 64 partitions with a tiny fp32 outer-product matmul
                    recf = normp.tile([1, 1024], F32, tag="recf", bufs=2)
                    nc.vector.reciprocal(out=recf[:, 0:512], in_=ctxu[64:65, 0:512])
                    nc.vector.reciprocal(
                        out=recf[:, 512:1024], in_=ctxu[64:65, 512:1024]
                    )
                    recb = normp.tile([1, 1024], BF16, tag="recb", bufs=2)
                    nc.vector.tensor_copy(out=recb, in_=recf)
                    # broadcast across 64 partitions via bf16 outer-product
                    # matmuls in the proj pool (keeps sc ping-pong slots free)
                    bc_a = proj_ps.tile([64, 512], F32, tag="proj")
                    bc_b = proj_ps.tile([64, 512], F32, tag="proj")
                    nc.tensor.matmul(
                        bc_a, lhsT=ones64b, rhs=recb[:, 0:512],
                        start=True, stop=True,
                    )
                    nc.tensor.matmul(
                        bc_b, lhsT=ones64b, rhs=recb[:, 512:1024],
                        start=True, stop=True,
                    )
                    bc_sb = normp.tile([64, 1024], BF16, tag="bc", bufs=2)
                    nc.vector.tensor_copy(out=bc_sb[:, 0:512], in_=bc_a)
                    nc.vector.tensor_copy(out=bc_sb[:, 512:1024], in_=bc_b)
                    nc.vector.tensor_mul(
                        out=ctxn[0:64, j, tsl],
                        in0=ctxu[0:64, 0:512],
                        in1=bc_sb[:, 0:512],
                    )
                    ctxn_b = normp.tile([64, 512], BF16, tag="ctxnb", bufs=2)
                    nc.vector.tensor_mul(
                        out=ctxn_b, in0=ctxu[0:64, 512:1024], in1=bc_sb[:, 512:1024]
                    )
                    nc.sync.dma_start(out=ctxn[64:128, j, tsl], in_=ctxn_b)

                # ---------------- output projection for this t-block -----
                for o in range(8):
                    ps = proj_ps.tile([128, 512], F32, tag="proj")
                    for c in range(4):
                        nc.tensor.matmul(
                            ps,
                            lhsT=woT[:, c, o * 128 : (o + 1) * 128],
                            rhs=ctxn[:, c, tsl],
                            start=(c == 0),
                            stop=(c == 3),
                        )
                    osb = osbp.tile([128, 512], F32, tag="osb")
                    nc.vector.tensor_copy(out=osb, in_=ps)
                    nc.sync.dma_start(
                        out=outd[o * 128 : (o + 1) * 128, tsl], in_=osb
                    )

    legalize_waits(nc)
    return nc


def _make_in_maps(inputs):
    q, k, v = inputs["q"], inputs["k"], inputs["v"]
    in_maps = []
    for c in range(N_CORES):
        b, hh = c // 2, c % 2
        esl = slice(hh * EL, (hh + 1) * EL)
        in_maps.append(
            {
                "q": np.ascontiguousarray(q[b], dtype=np.float32),
                "k": np.ascontiguousarray(k[b], dtype=np.float32),
                "v": np.ascontiguousarray(v[b], dtype=np.float32),
                "wq": np.ascontiguousarray(inputs["Wq"][esl], dtype=np.float32),
                "wk": np.ascontiguousarray(inputs["Wk"][esl], dtype=np.float32),
                "wv": np.ascontiguousarray(inputs["Wv"][esl], dtype=np.float32),
                "wo": np.ascontiguousarray(inputs["Wo"][:, esl], dtype=np.float32),
                "bq": np.ascontiguousarray(inputs["bq"][esl], dtype=np.float32),
                "bk": np.ascontiguousarray(inputs["bk"][esl], dtype=np.float32),
                "bv": np.ascontiguousarray(inputs["bv"][esl], dtype=np.float32),
            }
        )
    return in_maps


def _gather(results, bo):
    out = np.empty((B, T, E), dtype=np.float32)
    for b in range(B):
        acc = results[2 * b]["outT"].T + results[2 * b + 1]["outT"].T
        out[b] = acc + bo[None, :]
    return out


def run(inputs, **spmd_kwargs):
    if "nc" not in _CACHED:
        _CACHED["nc"] = build_program()
    nc = _CACHED["nc"]
    in_maps = _make_in_maps(inputs)
    res = run_bass_kernel_spmd(nc, in_maps, core_ids=list(range(N_CORES)), **spmd_kwargs)
    out = _gather(res.results, np.asarray(inputs["bo"], dtype=np.float32))
    return out, res


def kernel(**inputs) -> np.ndarray:
    out, _ = run(inputs)
    return out



# revision 28
# speedup vs baseline: 1.0296x; 1.0296x over previous
"""Multi-head attention (B=4, T=S=2048, E=1024, H=16) on 8 trn2 NeuronCores.

Sharding: core c handles batch b = c // 2 and head-half hh = c % 2
(8 of 16 heads).  Each core computes its heads' Q/K/V projections,
attention, and a partial output projection (contraction over its 512
e-dims).  The host sums the two partial outputs per batch and adds bo.

Pipeline design (cost-model driven):
 - ACT (exp over the full [s,t] score matrix) is the binding engine at
   ~266us; everything else is scheduled to hide beneath it.
 - Q/K projections and scores run in fp8e4m3 with DoubleRow matmuls.
   Projections contract E=1024 as 4 matmuls over (128 partitions x 2
   k-tiles); Wq/Wk/bq/bk are pre-scaled by 16 on the host so the fp8
   weight quantization stays out of the subnormal range.  Scores use a
   stride-0 k-tile dim (both k-tiles read the same 64 hd values, so
   the matmul computes 2x the score).  The combined 2*16*16 factor is
   folded into the exp scale.  fp8 q/k/score noise washes out in the
   softmax average; the v path stays bf16 since its error lands
   directly in the output.
 - PV is flipped: out tile [128t, 64d] per (head, t-subchunk), psum-
   accumulated over all 16 s-chunks; denominators come from 1-row
   matmuls (lhsT = exp-scores tile, rhs = ones); softmax normalization
   is a per-partition tensor_scalar multiply during the psum drain.
 - Attention iterates j (head-pair) OUTER, t-block inner, s-chunk
   innermost.  Scores/exp for iteration s+1 are emitted before the
   dependent den/PV work of iteration s (one-iteration skew) so sem
   waits never block the in-order PE queue ahead of the exp stream.
 - K/V/Q staging, the remaining projections, the output projection and
   ctx transposes are emitted as paced filler between attention
   iterations; PV matmuls trail their v-chunk production through a
   pending queue (bounded by the pt pool depth).
"""

from collections import deque

import numpy as np

import concourse.bass as bass
import concourse.mybir as mybir
import concourse.tile as tile
from concourse.bass_utils import run_bass_kernel_spmd
from concourse.masks import make_identity

F32 = mybir.dt.float32
BF16 = mybir.dt.bfloat16
FP8 = mybir.dt.float8e4

B, T, E = 4, 2048, 1024
H = 16  # global heads
HL = 8  # heads per core (local)
HD = 64  # head dim
EL = HL * HD  # 512, e-dims per core
N_CORES = 8
DR = mybir.MatmulPerfMode.DoubleRow
WSCALE = 16.0  # host-side pre-scale of Wq/Wk (and bq/bk)
EXP_SCALE = 0.125 / (WSCALE * WSCALE)  # BISECT: no-DR scores (1x)

_CACHED = {}


def legalize_waits(nc, cap=1):
    """Hoist semaphore waits so no instruction carries more than `cap`.

    The cayman 64B ISA instruction format has a single wait slot
    (NEURON_ISA_TPB_EVENTS); this container's walrus rejects instructions
    with more attached waits ("Too many sync wait commands").  Tile's sem
    assignment freely attaches several, so we split the excess onto
    standalone InstEventSemaphore carriers (exactly what raw-bass
    wait_ge emits) on the same engine, immediately before.
    """
    import bass_rust

    totals = {}
    names = {}
    for f in nc.m.functions:
        for bb in f.blocks:
            for ins in bb.instructions:
                si = ins.sync_info
                if si is None:
                    continue
                for u in si.on_update or []:
                    if u.sync_type == "semaphore":
                        sign = 1 if u.update_mode in ("sem-inc", "sem-add-imm") else -1
                        totals[u.id] = totals.get(u.id, 0) + sign * u.update_value
                        names[u.id] = u.ant_name

    n = 0
    for f in nc.m.functions:
        for bb in f.blocks:
            insts = bb.instructions
            out = []
            changed = False
            for ins in insts:
                if type(ins).__name__ == "InstISA" and "RANGE_CLEAR" in str(ins):
                    import re

                    m = re.search(r"range_first=(\d+) range_last=(\d+)", str(ins))
                    first, last = int(m.group(1)), int(m.group(2))
                    for sid in range(first, last + 1):
                        tot = totals.get(sid, 0)
                        if tot == 0:
                            continue
                        ev = mybir.InstEventSemaphore(name=f"I-LC{n}", ins=[], outs=[])
                        n += 1
                        ev.engine = ins.engine
                        ev.sync_info = bass_rust.SyncInfo(
                            on_wait=[],
                            on_update=[
                                bass_rust.SyncUpdate(
                                    sync_type="semaphore",
                                    id=sid,
                                    ant_name=names.get(sid, f"sem{sid}"),
                                    update_mode="sem-sub-imm",
                                    update_value=tot,
                                    update_reg=None,
                                )
                            ],
                        )
                        out.append(ev)
                    changed = True
                    continue
                si = ins.sync_info
                ws = list(si.on_wait) if (si is not None and si.on_wait) else []
                if len(ws) > cap:
                    for w in ws[: len(ws) - cap]:
                        ev = mybir.InstEventSemaphore(
                            name=f"I-LW{n}", ins=[], outs=[]
                        )
                        n += 1
                        ev.engine = ins.engine
                        ev.sync_info = bass_rust.SyncInfo(
                            on_wait=[w], on_update=[]
                        )
                        out.append(ev)
                    si.on_wait = ws[len(ws) - cap :]
                    changed = True
                out.append(ins)
            if changed:
                insts[:] = out
    return n


def build_program():
    nc = bass.Bass()

    qd = nc.declare_dram_parameter("q", [T, E], F32, isOutput=False)
    kd = nc.declare_dram_parameter("k", [T, E], F32, isOutput=False)
    vd = nc.declare_dram_parameter("v", [T, E], F32, isOutput=False)
    wqd = nc.declare_dram_parameter("wq", [EL, E], F32, isOutput=False)
    wkd = nc.declare_dram_parameter("wk", [EL, E], F32, isOutput=False)
    wvd = nc.declare_dram_parameter("wv", [EL, E], F32, isOutput=False)
    wod = nc.declare_dram_parameter("wo", [E, EL], F32, isOutput=False)
    bqd = nc.declare_dram_parameter("bq", [EL], F32, isOutput=False)
    bkd = nc.declare_dram_parameter("bk", [EL], F32, isOutput=False)
    bvd = nc.declare_dram_parameter("bv", [EL], F32, isOutput=False)
    outd = nc.declare_dram_parameter("outT", [E, T], F32, isOutput=True)
    dbg_qp = nc.declare_dram_parameter("dbg_qp", [128, 4, T], F32, isOutput=True)
    dbg_kp = nc.declare_dram_parameter("dbg_kp", [128, 4, T], F32, isOutput=True)
    dbg_vp = nc.declare_dram_parameter("dbg_vp", [128, 16, EL], F32, isOutput=True)
    dbg_cx = nc.declare_dram_parameter("dbg_cx", [128, 4, EL], F32, isOutput=True)
    dbg_pt = nc.declare_dram_parameter("dbg_pt", [16, 128, 1024], F32, isOutput=True)
    dbg_rec = nc.declare_dram_parameter("dbg_rec", [128, 8], F32, isOutput=True)

    with tile.TileContext(nc, pool_alloc_mode="queue") as tc:
        with (
            tc.tile_pool(name="singles", bufs=1) as singles,
            tc.tile_pool(name="stage", bufs=2) as stage,
            tc.tile_pool(name="wstage", bufs=2) as wstage,
            tc.tile_pool(name="xq", bufs=2) as xqp,
            tc.tile_pool(name="xv", bufs=2) as xvp,
            tc.tile_pool(name="pt", bufs=10) as ptp,
            tc.tile_pool(name="rec", bufs=2) as recp,
            tc.tile_pool(name="ctxn", bufs=4) as ctxnp,
            tc.tile_pool(name="ctxT", bufs=2) as ctxTp,
            tc.tile_pool(name="osb", bufs=2) as osbp,
            tc.tile_pool(name="sc_ps", bufs=2, space="PSUM") as sc_ps,
            tc.tile_pool(name="ctx_ps", bufs=1, space="PSUM") as ctx_ps,
            tc.tile_pool(name="den_ps", bufs=1, space="PSUM") as den_ps,
            tc.tile_pool(name="work_ps", bufs=2, space="PSUM") as work_ps,
        ):
            # ---------------- constants ----------------------------------
            ident = singles.tile([128, 128], BF16)
            make_identity(nc, ident)
            ones1 = singles.tile([128, 1], BF16)
            nc.vector.memset(ones1, 1.0)
            ones_row = singles.tile([1, 128], BF16)
            nc.vector.memset(ones_row, 1.0)

            bq_sb = singles.tile([128, 4], F32)
            bk_sb = singles.tile([128, 4], F32)
            bv_sb = singles.tile([1, EL], BF16)
            nc.gpsimd.dma_start(out=bq_sb, in_=bqd.rearrange("(c p) -> p c", p=128))
            nc.gpsimd.dma_start(out=bk_sb, in_=bkd.rearrange("(c p) -> p c", p=128))
            nc.gpsimd.dma_start(out=bv_sb, in_=bvd.rearrange("(o e) -> o e", o=1))

            # transposed weights: q/k in fp8 (DoubleRow projections), v/o bf16
            wqT = singles.tile([128, 8, EL], FP8)
            wkT = singles.tile([128, 8, EL], FP8)
            wvT = singles.tile([128, 8, EL], BF16)
            woT = singles.tile([128, 4, E], BF16)

            # persistent activations
            kT = singles.tile([128, 8, T], FP8)  # kT[p, e, t] = k[t, e*128+p]
            qp8 = singles.tile([128, 4, T], FP8)  # qp8[p, j, t] (x WSCALE)
            kp8 = singles.tile([128, 4, T], FP8)
            vp = singles.tile([128, 16, EL], BF16)  # vp[p, sc, e]

            # ---------------- emission helpers ---------------------------
            def load_cast(xd, nrows, pool, tag):
                a = nrows // 128
                ncols = xd.shape[1]
                xb = pool.tile([128, a, ncols], BF16, tag=tag)
                nc.gpsimd.dma_start(
                    out=xb, in_=xd.rearrange("(a p) e -> p a e", p=128)
                )
                return xb

            def transpose_chunk(dst_view, src, e, a_chunks):
                """dst_view <- transposes of src[:, a, e*128:+128] (cast on copy)."""
                n = a_chunks * 128
                wk = work_ps.tile([128, 512], F32, tag="work")
                tr = wk.bitcast(BF16)
                for a in range(a_chunks):
                    nc.tensor.transpose(
                        tr[:, a * 128 : (a + 1) * 128],
                        src[:, a, e * 128 : (e + 1) * 128],
                        ident,
                    )
                nc.vector.tensor_copy(out=dst_view, in_=tr[:, 0:n])

            def w_transpose(wd, wT, a_chunks, e_chunks):
                wb = load_cast(wd, a_chunks * 128, wstage, "wstage")
                for e in range(e_chunks):
                    transpose_chunk(wT[:, e, :], wb, e, a_chunks)

            def qk_proj(xT8, xoff, wT8, b_sb, xp8, j, tb):
                """fp8 DoubleRow projection: one [128e, 512t] chunk + bias cast."""
                ps = work_ps.tile([128, 512], F32, tag="work")
                for i in range(8):
                    nc.tensor.matmul(
                        ps,
                        lhsT=wT8[:, i, j * 128 : (j + 1) * 128],
                        rhs=xT8[:, i, xoff : xoff + 512],
                        start=(i == 0),
                        stop=(i == 7),
                    )
                nc.vector.tensor_scalar_add(
                    out=xp8[:, j, tb * 512 : (tb + 1) * 512],
                    in0=ps,
                    scalar1=b_sb[:, j : j + 1],
                )

            def v_proj(vT_blk, s):
                ps = work_ps.tile([128, 512], F32, tag="work")
                for e in range(8):
                    nc.tensor.matmul(
                        ps,
                        lhsT=vT_blk[:, e, (s % 4) * 128 : (s % 4 + 1) * 128],
                        rhs=wvT[:, e, :],
                        start=(e == 0),
                        stop=False,
                    )
                nc.tensor.matmul(ps, lhsT=ones_row, rhs=bv_sb, start=False, stop=True)
                nc.vector.tensor_copy(out=vp[:, s, :], in_=ps)

            # ---------------- filler / pending machinery ------------------
            state = {"produced_v": 4, "credit": 0.0, "n_emitted": 0}
            fill_q = deque()  # (rows, fn), single deadline-ordered queue
            pend_q = deque()  # (need_v, fn): PV/normalize closures
            marks = {}

            def drain_pend():
                while pend_q and pend_q[0][0] <= state["produced_v"]:
                    pend_q.popleft()[1]()

            def pump(gain=0.0, flush=False):
                state["credit"] = min(state["credit"] + gain, 5600.0)
                while fill_q and (flush or fill_q[0][0] <= state["credit"]):
                    rows, fn = fill_q.popleft()
                    fn()
                    state["n_emitted"] += 1
                    if not flush:
                        state["credit"] -= rows
                    drain_pend()
                drain_pend()

            def ensure(mark):
                need = marks.get(mark, 0)
                while state["n_emitted"] < need and fill_q:
                    rows, fn = fill_q.popleft()
                    fn()
                    state["n_emitted"] += 1
                    drain_pend()

            def pend_guard(maxlen=8):
                """Bound PV trailing so pt pool slots are never re-allocated
                before their pending reader is emitted (pt bufs=18 > maxlen+1).
                PV may trail up to a whole block; den half ping-pong stays
                safe because norm(g) must drain before block g+2 begins."""
                while len(pend_q) > maxlen:
                    if pend_q[0][0] <= state["produced_v"]:
                        pend_q.popleft()[1]()
                    elif fill_q:
                        rows, fn = fill_q.popleft()
                        fn()
                        state["n_emitted"] += 1
                        drain_pend()
                    else:
                        break

            # ---------------- prologue ------------------------------------
            w_transpose(wkd, wkT, 4, 8)
            w_transpose(wqd, wqT, 4, 8)

            kb0 = load_cast(kd[0:512, :], 512, stage, "xstage")
            for e in range(8):
                transpose_chunk(kT[:, e, 0:512], kb0, e, 4)
            qk_proj(kT, 0, wkT, bk_sb, kp8, 0, 0)

            qb0 = load_cast(qd[0:512, :], 512, stage, "xstage")
            qT0 = xqp.tile([128, 8, 512], FP8, tag="qT")
            for e in range(8):
                transpose_chunk(qT0[:, e, :], qb0, e, 4)
            for jj in range(4):
                qk_proj(qT0, 0, wqT, bq_sb, qp8, jj, 0)

            w_transpose(wvd, wvT, 4, 8)
            vb0 = load_cast(vd[0:512, :], 512, stage, "xstage")
            vT0 = xvp.tile([128, 8, 512], BF16, tag="vT")
            for e in range(8):
                transpose_chunk(vT0[:, e, :], vb0, e, 4)
            for s in range(4):
                v_proj(vT0, s)

            # ---------------- filler units --------------------------------
            staged = {}

            def prefetch(xd, blk, key):
                staged[key] = load_cast(
                    xd[blk * 512 : (blk + 1) * 512, :], 512, stage, "xstage"
                )

            # All remaining input DMAs, queued now in consumption order; the
            # stage pool ring (bufs=4) provides backpressure so ~2 stay in
            # flight ahead of their consumers and none sits on the critical
            # path of a forced staging chain.
            for _xd, _blk, _key in (
                (kd, 1, ("k", 1)),
                (kd, 2, ("k", 2)),
                (kd, 3, ("k", 3)),
                (vd, 1, ("v", 1)),
                (qd, 1, ("q", 1)),
                (vd, 2, ("v", 2)),
                (qd, 2, ("q", 2)),
                (vd, 3, ("v", 3)),
                (qd, 3, ("q", 3)),
            ):
                prefetch(_xd, _blk, _key)

            def mk_xtr1(src_key, dst_fn, e, pop):
                """Transpose ONE e-chunk of a staged x block (512 rows + 1 copy)."""
                def f():
                    xb = staged[src_key]
                    transpose_chunk(dst_fn(e), xb, e, 4)
                    if pop:
                        staged.pop(src_key)

                return f

            def mk_kblk_units(blk):
                dst = lambda e: kT[:, e, blk * 512 : (blk + 1) * 512]
                u = [(512, mk_xtr1(("k", blk), dst, e, e == 7)) for e in range(8)]
                u.append((1024, lambda: qk_proj(kT, blk * 512, wkT, bk_sb, kp8, 0, blk)))
                return u

            def mk_valloc(blk):
                def f():
                    vT = xvp.tile([128, 8, 512], BF16, tag="vT")
                    staged[("vT", blk)] = vT

                return f

            def mk_vblk_units(blk):
                def dst(e):
                    return staged[("vT", blk)][:, e, :]

                u = [(1, mk_valloc(blk))]
                u += [(512, mk_xtr1(("v", blk), dst, e, e == 7)) for e in range(8)]
                return u

            def mk_vproj(blk, s):
                def f():
                    v_proj(staged[("vT", blk)], s)
                    state["produced_v"] = s + 1

                return f

            def mk_qalloc(tb):
                def f():
                    qT = xqp.tile([128, 8, 512], FP8, tag="qT")
                    staged[("qT", tb)] = qT

                return f

            def mk_qT_units(tb):
                def dst(e):
                    return staged[("qT", tb)][:, e, :]

                u = [(1, mk_qalloc(tb))]
                u += [(512, mk_xtr1(("q", tb), dst, e, e == 7)) for e in range(8)]
                return u

            def mk_qproj(j, tb):
                return (1024, lambda: qk_proj(staged[("qT", tb)], 0, wqT, bq_sb, qp8, j, tb))

            def mk_kproj(j, blk):
                return (1024, lambda: qk_proj(kT, blk * 512, wkT, bk_sb, kp8, j, blk))

            def mk_wo_units():
                def fload():
                    staged["wo"] = load_cast(wod, E, wstage, "wstage")

                def mk_tr(e):
                    return lambda: transpose_chunk(woT[:, e, :], staged["wo"], e, 8)

                u = [(1, fload)]
                u += [(1024, mk_tr(e)) for e in range(4)]
                return u

            # single queue, deadline-ordered, v staging interleaved early so
            # PV-pend lag (cap 16) absorbs the production/consumption gap.
            for u in mk_kblk_units(1):
                fill_q.append(u)
            marks[("kblk", 1)] = len(fill_q)
            for u in mk_kblk_units(2):
                fill_q.append(u)
            marks[("kblk", 2)] = len(fill_q)
            for u in mk_vblk_units(1):
                fill_q.append(u)
            fill_q.append((4608, mk_vproj(1, 4)))
            for u in mk_kblk_units(3):
                fill_q.append(u)
            marks[("kblk", 3)] = len(fill_q)
            for s in range(5, 8):
                fill_q.append((4608, mk_vproj(1, s)))
            for blk in range(4):
                fill_q.append(mk_kproj(1, blk))
            marks[(1, 0)] = len(fill_q)
            for u in mk_vblk_units(2):
                fill_q.append(u)
            fill_q.append((4608, mk_vproj(2, 8)))
            fill_q.append((4608, mk_vproj(2, 9)))
            for u in mk_qT_units(1):
                fill_q.append(u)
            for jj in range(4):
                fill_q.append(mk_qproj(jj, 1))
            marks[(0, 1)] = len(fill_q)
            fill_q.append((4608, mk_vproj(2, 10)))
            fill_q.append((4608, mk_vproj(2, 11)))
            for u in mk_vblk_units(3):
                fill_q.append(u)
            fill_q.append((4608, mk_vproj(3, 12)))
            fill_q.append((4608, mk_vproj(3, 13)))
            for u in mk_qT_units(2):
                fill_q.append(u)
            for jj in range(4):
                fill_q.append(mk_qproj(jj, 2))
            marks[(0, 2)] = len(fill_q)
            fill_q.append((4608, mk_vproj(3, 14)))
            fill_q.append((4608, mk_vproj(3, 15)))
            for u in mk_qT_units(3):
                fill_q.append(u)
            for jj in range(4):
                fill_q.append(mk_qproj(jj, 3))
            marks[(0, 3)] = len(fill_q)
            for blk in range(4):
                fill_q.append(mk_kproj(2, blk))
            marks[(2, 3)] = len(fill_q)
            for blk in range(4):
                fill_q.append(mk_kproj(3, blk))
            marks[(3, 3)] = len(fill_q)
            for u in mk_wo_units():
                fill_q.append(u)

            # ---------------- attention -----------------------------------
            den_t = den_ps.tile([128, 16], F32, tag="den")
            denA = den_t[:, 0:8]
            denB = den_t[:, 8:16]

            def attention_block(j, tb, gidx):
                tsl = slice(tb * 512, (tb + 1) * 512)
                den = denA if gidx % 2 == 0 else denB
                # start=True on any matmul wipes co-resident accumulation
                # groups in the same PSUM bank, so zero the region once and
                # accumulate with start=False throughout.
                nc.vector.memset(den, 0.0)
                pts = {}

                def emit_scores(s):
                    ssl = slice(s * 128, (s + 1) * 128)
                    sc = sc_ps.tile([128, 1024], F32, tag="sc")
                    for h in range(2):
                        hp = slice(h * 64, (h + 1) * 64)
                        nc.tensor.matmul(
                            sc[:, h * 512 : (h + 1) * 512],
                            lhsT=kp8[hp, j, ssl],
                            rhs=qp8[hp, j, tsl],
                            start=True,
                            stop=True,
                        )
                    pt = ptp.tile([128, 1024], BF16, tag="pt")
                    nc.scalar.activation(
                        out=pt,
                        in_=sc,
                        func=mybir.ActivationFunctionType.Exp,
                        scale=EXP_SCALE,
                    )
                    pts[s] = pt
                    if gidx == 0:
                        nc.gpsimd.dma_start(out=dbg_pt[s], in_=pt)

                def emit_den(s):
                    pt = pts[s]
                    for h in range(2):
                        for tc in range(4):
                            nc.tensor.matmul(
                                den[:, h * 4 + tc : h * 4 + tc + 1],
                                lhsT=pt[:, h * 512 + tc * 128 : h * 512 + tc * 128 + 128],
                                rhs=ones1,
                                start=False,
                                stop=(s == 15),
                                skip_group_check=True,
                            )

                # ctx tile is allocated lazily by the first pv closure so the
                # 1-buf pool rotation lands in pend order.
                box = {}

                def mk_pv(s):
                    def f():
                        if "ctx" not in box:
                            ctx = ctx_ps.tile([128, 512], F32, tag="ctx")
                            box["ctx"] = ctx
                            nc.vector.memset(ctx, 0.0)
                        ctx = box["ctx"]
                        pt = pts.pop(s)
                        for h in range(2):
                            for tc in range(4):
                                nc.tensor.matmul(
                                    ctx[:, (h * 4 + tc) * 64 : (h * 4 + tc) * 64 + 64],
                                    lhsT=pt[:, h * 512 + tc * 128 : h * 512 + tc * 128 + 128],
                                    rhs=vp[:, s, (2 * j + h) * 64 : (2 * j + h + 1) * 64],
                                    start=False,
                                    stop=(s == 15),
                                    skip_group_check=True,
                                )

                    return f

                def mk_norm():
                    def f():
                        ctx = box["ctx"]
                        rec = recp.tile([128, 8], F32, tag="rec")
                        nc.vector.reciprocal(out=rec, in_=den)
                        if gidx == 0:
                            nc.sync.dma_start(out=dbg_rec[:, :], in_=rec)
                        ctxn = state["ctxn"][tb]
                        for h in range(2):
                            for tc in range(4):
                                nc.vector.tensor_scalar_mul(
                                    out=ctxn[:, tc, (2 * j + h) * 64 : (2 * j + h + 1) * 64],
                                    in0=ctx[:, (h * 4 + tc) * 64 : (h * 4 + tc) * 64 + 64],
                                    scalar1=rec[:, h * 4 + tc : h * 4 + tc + 1],
                                )

                    return f

                emit_scores(0)
                for s in range(16):
                    pend_guard()
                    if s + 1 < 16:
                        if gidx == 0 and (s + 1) % 4 == 0:
                            ensure(("kblk", (s + 1) // 4))
                        emit_scores(s + 1)
                    emit_den(s)
                    pend_q.append((s + 1, mk_pv(s)))
                    pump(1400.0)
                pend_q.append((16, mk_norm()))
                drain_pend()

            def mk_ctxT_outproj(tb):
                """Emitted as filler after (j3, tb): transpose ctx, project out."""
                ctxn = state["ctxn"][tb]
                ctxT = ctxTp.tile([128, 4, 512], BF16, tag="ctxT")

                def mk_tr(ec):
                    def f():
                        wk = work_ps.tile([128, 512], F32, tag="work")
                        tr = wk.bitcast(BF16)
                        for tc in range(4):
                            nc.tensor.transpose(
                                tr[:, tc * 128 : (tc + 1) * 128],
                                ctxn[:, tc, ec * 128 : (ec + 1) * 128],
                                ident,
                            )
                        nc.vector.tensor_copy(out=ctxT[:, ec, :], in_=tr[:, 0:512])

                    return f

                def mk_out(o):
                    def f():
                        ps = work_ps.tile([128, 512], F32, tag="work")
                        for c in range(4):
                            nc.tensor.matmul(
                                ps,
                                lhsT=woT[:, c, o * 128 : (o + 1) * 128],
                                rhs=ctxT[:, c, :],
                                start=(c == 0),
                                stop=(c == 3),
                            )
                        osb = osbp.tile([128, 512], F32, tag="osb")
                        nc.vector.tensor_copy(out=osb, in_=ps)
                        nc.sync.dma_start(
                            out=outd[o * 128 : (o + 1) * 128, tb * 512 : (tb + 1) * 512],
                            in_=osb,
                        )

                    return f

                units = [(512, mk_tr(ec)) for ec in range(4)]
                units += [(2048, mk_out(o)) for o in range(8)]
                return units

            state["ctxn"] = {}
            for tb in range(4):
                ct = ctxnp.tile([128, 4, 512], BF16, tag="ctxn")
                state["ctxn"][tb] = ct

            BLOCKS = [
                (0, 0), (1, 0), (0, 1), (1, 1), (0, 2), (1, 2), (0, 3), (1, 3),
                (2, 3), (3, 3), (2, 0), (3, 0), (2, 1), (3, 1), (2, 2), (3, 2),
            ]
            for gidx, (j, tb) in enumerate(BLOCKS):
                ensure((j, tb))
                attention_block(j, tb, gidx)
                if j == 3:
                    # ctx for this t-block complete: queue its output projection
                    for u in mk_ctxT_outproj(tb):
                        fill_q.append(u)
            pump(flush=True)
            drain_pend()
            nc.gpsimd.dma_start(out=dbg_qp[:, :, :], in_=qp8)
            nc.gpsimd.dma_start(out=dbg_kp[:, :, :], in_=kp8)
            nc.gpsimd.dma_start(out=dbg_vp[:, :, :], in_=vp)
            nc.gpsimd.dma_start(out=dbg_cx[:, :, :], in_=state["ctxn"][0])

    legalize_waits(nc)
    return nc


def _make_in_maps(inputs):
    q, k, v = inputs["q"], inputs["k"], inputs["v"]
    in_maps = []
    for c in range(N_CORES):
        b, hh = c // 2, c % 2
        esl = slice(hh * EL, (hh + 1) * EL)
        in_maps.append(
            {
                "q": np.ascontiguousarray(q[b], dtype=np.float32),
                "k": np.ascontiguousarray(k[b], dtype=np.float32),
                "v": np.ascontiguousarray(v[b], dtype=np.float32),
                "wq": np.ascontiguousarray(inputs["Wq"][esl] * WSCALE, dtype=np.float32),
                "wk": np.ascontiguousarray(inputs["Wk"][esl] * WSCALE, dtype=np.float32),
                "wv": np.ascontiguousarray(inputs["Wv"][esl], dtype=np.float32),
                "wo": np.ascontiguousarray(inputs["Wo"][:, esl], dtype=np.float32),
                "bq": np.ascontiguousarray(inputs["bq"][esl] * WSCALE, dtype=np.float32),
                "bk": np.ascontiguousarray(inputs["bk"][esl] * WSCALE, dtype=np.float32),
                "bv": np.ascontiguousarray(inputs["bv"][esl], dtype=np.float32),
            }
        )
    return in_maps


def _gather(results, bo):
    out = np.empty((B, T, E), dtype=np.float32)
    for b in range(B):
        acc = results[2 * b]["outT"].T + results[2 * b + 1]["outT"].T
        out[b] = acc + bo[None, :]
    return out


def run(inputs, **spmd_kwargs):
    if "nc" not in _CACHED:
        _CACHED["nc"] = build_program()
    nc = _CACHED["nc"]
    in_maps = _make_in_maps(inputs)
    res = run_bass_kernel_spmd(nc, in_maps, core_ids=list(range(N_CORES)), **spmd_kwargs)
    out = _gather(res.results, np.asarray(inputs["bo"], dtype=np.float32))
    return out, res


def kernel(**inputs) -> np.ndarray:
    out, _ = run(inputs)
    return out


# revision 29
# speedup vs baseline: 1.1340x; 1.1015x over previous
"""Multi-head attention (B=4, T=S=2048, E=1024, H=16) on 8 trn2 NeuronCores.

Sharding: core c handles batch b = c // 2 and head-half hh = c % 2
(8 of 16 heads).  Each core computes its heads' Q/K/V projections,
attention, and a partial output projection (contraction over its 512
e-dims).  The host sums the two partial outputs per batch and adds bo.

Pipeline design (cost-model driven):
 - ACT (exp over the full [s,t] score matrix) is the binding engine at
   ~266us; everything else is scheduled to hide beneath it.
 - Q/K projections and scores run in fp8e4m3 with DoubleRow matmuls.
   Projections contract E=1024 as 4 matmuls over (128 partitions x 2
   k-tiles); Wq/Wk/bq/bk are pre-scaled by 16 on the host so the fp8
   weight quantization stays out of the subnormal range.  Scores use a
   stride-0 k-tile dim (both k-tiles read the same 64 hd values, so
   the matmul computes 2x the score).  The combined 2*16*16 factor is
   folded into the exp scale.  fp8 q/k/score noise washes out in the
   softmax average; the v path stays bf16 since its error lands
   directly in the output.
 - PV is flipped: out tile [128t, 64d] per (head, t-subchunk), psum-
   accumulated over all 16 s-chunks; denominators come from 1-row
   matmuls (lhsT = exp-scores tile, rhs = ones); softmax normalization
   is a per-partition tensor_scalar multiply during the psum drain.
 - Attention iterates j (head-pair) OUTER, t-block inner, s-chunk
   innermost.  Scores/exp for iteration s+1 are emitted before the
   dependent den/PV work of iteration s (one-iteration skew) so sem
   waits never block the in-order PE queue ahead of the exp stream.
 - K/V/Q staging, the remaining projections, the output projection and
   ctx transposes are emitted as paced filler between attention
   iterations; PV matmuls trail their v-chunk production through a
   pending queue (bounded by the pt pool depth).
"""

from collections import deque

import numpy as np

import concourse.bass as bass
import concourse.mybir as mybir
import concourse.tile as tile
from concourse.bass_utils import run_bass_kernel_spmd
from concourse.masks import make_identity

F32 = mybir.dt.float32
BF16 = mybir.dt.bfloat16
FP8 = mybir.dt.float8e4

B, T, E = 4, 2048, 1024
H = 16  # global heads
HL = 8  # heads per core (local)
HD = 64  # head dim
EL = HL * HD  # 512, e-dims per core
N_CORES = 8
DR = mybir.MatmulPerfMode.DoubleRow
WSCALE = 16.0  # host-side pre-scale of Wq/Wk (and bq/bk)
EXP_SCALE = 0.0625 / (WSCALE * WSCALE)  # 1/sqrt(hd) / (2 * 16 * 16)

_CACHED = {}


def legalize_waits(nc, cap=1):
    """Hoist semaphore waits so no instruction carries more than `cap`.

    The cayman 64B ISA instruction format has a single wait slot
    (NEURON_ISA_TPB_EVENTS); this container's walrus rejects instructions
    with more attached waits ("Too many sync wait commands").  Tile's sem
    assignment freely attaches several, so we split the excess onto
    standalone InstEventSemaphore carriers (exactly what raw-bass
    wait_ge emits) on the same engine, immediately before.
    """
    import bass_rust

    totals = {}
    names = {}
    for f in nc.m.functions:
        for bb in f.blocks:
            for ins in bb.instructions:
                si = ins.sync_info
                if si is None:
                    continue
                for u in si.on_update or []:
                    if u.sync_type == "semaphore":
                        sign = 1 if u.update_mode in ("sem-inc", "sem-add-imm") else -1
                        totals[u.id] = totals.get(u.id, 0) + sign * u.update_value
                        names[u.id] = u.ant_name

    n = 0
    for f in nc.m.functions:
        for bb in f.blocks:
            insts = bb.instructions
            out = []
            changed = False
            for ins in insts:
                if type(ins).__name__ == "InstISA" and "RANGE_CLEAR" in str(ins):
                    import re

                    m = re.search(r"range_first=(\d+) range_last=(\d+)", str(ins))
                    first, last = int(m.group(1)), int(m.group(2))
                    for sid in range(first, last + 1):
                        tot = totals.get(sid, 0)
                        if tot == 0:
                            continue
                        ev = mybir.InstEventSemaphore(name=f"I-LC{n}", ins=[], outs=[])
                        n += 1
                        ev.engine = ins.engine
                        ev.sync_info = bass_rust.SyncInfo(
                            on_wait=[],
                            on_update=[
                                bass_rust.SyncUpdate(
                                    sync_type="semaphore",
                                    id=sid,
                                    ant_name=names.get(sid, f"sem{sid}"),
                                    update_mode="sem-sub-imm",
                                    update_value=tot,
                                    update_reg=None,
                                )
                            ],
                        )
                        out.append(ev)
                    changed = True
                    continue
                si = ins.sync_info
                ws = list(si.on_wait) if (si is not None and si.on_wait) else []
                if len(ws) > cap:
                    for w in ws[: len(ws) - cap]:
                        ev = mybir.InstEventSemaphore(
                            name=f"I-LW{n}", ins=[], outs=[]
                        )
                        n += 1
                        ev.engine = ins.engine
                        ev.sync_info = bass_rust.SyncInfo(
                            on_wait=[w], on_update=[]
                        )
                        out.append(ev)
                    si.on_wait = ws[len(ws) - cap :]
                    changed = True
                out.append(ins)
            if changed:
                insts[:] = out
    return n


def build_program():
    nc = bass.Bass()

    qd = nc.declare_dram_parameter("q", [T, E], F32, isOutput=False)
    kd = nc.declare_dram_parameter("k", [T, E], F32, isOutput=False)
    vd = nc.declare_dram_parameter("v", [T, E], F32, isOutput=False)
    wqd = nc.declare_dram_parameter("wq", [EL, E], F32, isOutput=False)
    wkd = nc.declare_dram_parameter("wk", [EL, E], F32, isOutput=False)
    wvd = nc.declare_dram_parameter("wv", [EL, E], F32, isOutput=False)
    wod = nc.declare_dram_parameter("wo", [E, EL], F32, isOutput=False)
    bqd = nc.declare_dram_parameter("bq", [EL], F32, isOutput=False)
    bkd = nc.declare_dram_parameter("bk", [EL], F32, isOutput=False)
    bvd = nc.declare_dram_parameter("bv", [EL], F32, isOutput=False)
    outd = nc.declare_dram_parameter("outT", [E, T], F32, isOutput=True)

    with tile.TileContext(nc, pool_alloc_mode="queue") as tc:
        with (
            tc.tile_pool(name="singles", bufs=1) as singles,
            tc.tile_pool(name="stage", bufs=2) as stage,
            tc.tile_pool(name="wstage", bufs=2) as wstage,
            tc.tile_pool(name="xq", bufs=2) as xqp,
            tc.tile_pool(name="xv", bufs=2) as xvp,
            tc.tile_pool(name="pt", bufs=10) as ptp,
            tc.tile_pool(name="rec", bufs=2) as recp,
            tc.tile_pool(name="ctxn", bufs=4) as ctxnp,
            tc.tile_pool(name="ctxT", bufs=2) as ctxTp,
            tc.tile_pool(name="osb", bufs=2) as osbp,
            tc.tile_pool(name="sc_ps", bufs=2, space="PSUM") as sc_ps,
            tc.tile_pool(name="ctx_ps", bufs=1, space="PSUM") as ctx_ps,
            tc.tile_pool(name="den_ps", bufs=1, space="PSUM") as den_ps,
            tc.tile_pool(name="work_ps", bufs=2, space="PSUM") as work_ps,
        ):
            # ---------------- constants ----------------------------------
            ident = singles.tile([128, 128], BF16)
            make_identity(nc, ident)
            ones1 = singles.tile([128, 1], BF16)
            nc.vector.memset(ones1, 1.0)
            ones_row = singles.tile([1, 128], BF16)
            nc.vector.memset(ones_row, 1.0)

            bq_sb = singles.tile([128, 4], F32)
            bk_sb = singles.tile([128, 4], F32)
            bv_sb = singles.tile([1, EL], BF16)
            nc.gpsimd.dma_start(out=bq_sb, in_=bqd.rearrange("(c p) -> p c", p=128))
            nc.gpsimd.dma_start(out=bk_sb, in_=bkd.rearrange("(c p) -> p c", p=128))
            nc.gpsimd.dma_start(out=bv_sb, in_=bvd.rearrange("(o e) -> o e", o=1))

            # transposed weights: q/k in fp8 (DoubleRow projections), v/o bf16
            wqT = singles.tile([128, 8, EL], FP8)
            wkT = singles.tile([128, 8, EL], FP8)
            wvT = singles.tile([128, 8, EL], BF16)
            woT = singles.tile([128, 4, E], BF16)

            # persistent activations
            kT = singles.tile([128, 8, T], FP8)  # kT[p, e, t] = k[t, e*128+p]
            qp8 = singles.tile([128, 4, T], FP8)  # qp8[p, j, t] (x WSCALE)
            kp8 = singles.tile([128, 4, T], FP8)
            vp = singles.tile([128, 16, EL], BF16)  # vp[p, sc, e]

            # ---------------- emission helpers ---------------------------
            def load_cast(xd, nrows, pool, tag):
                a = nrows // 128
                ncols = xd.shape[1]
                xb = pool.tile([128, a, ncols], BF16, tag=tag)
                nc.gpsimd.dma_start(
                    out=xb, in_=xd.rearrange("(a p) e -> p a e", p=128)
                )
                return xb

            def transpose_chunk(dst_view, src, e, a_chunks):
                """dst_view <- transposes of src[:, a, e*128:+128] (cast on copy)."""
                n = a_chunks * 128
                wk = work_ps.tile([128, 512], F32, tag="work")
                tr = wk.bitcast(BF16)
                for a in range(a_chunks):
                    nc.tensor.transpose(
                        tr[:, a * 128 : (a + 1) * 128],
                        src[:, a, e * 128 : (e + 1) * 128],
                        ident,
                    )
                nc.vector.tensor_copy(out=dst_view, in_=tr[:, 0:n])

            def w_transpose(wd, wT, a_chunks, e_chunks):
                wb = load_cast(wd, a_chunks * 128, wstage, "wstage")
                for e in range(e_chunks):
                    transpose_chunk(wT[:, e, :], wb, e, a_chunks)

            def qk_proj(xT8, xoff, wT8, b_sb, xp8, j, tb):
                """fp8 DoubleRow projection: one [128e, 512t] chunk + bias cast."""
                ps = work_ps.tile([128, 512], F32, tag="work")
                for i in range(4):
                    nc.tensor.matmul(
                        ps,
                        lhsT=wT8[:, 2 * i : 2 * i + 2, j * 128 : (j + 1) * 128],
                        rhs=xT8[:, 2 * i : 2 * i + 2, xoff : xoff + 512],
                        start=(i == 0),
                        stop=(i == 3),
                        perf_mode=DR,
                    )
                nc.vector.tensor_scalar_add(
                    out=xp8[:, j, tb * 512 : (tb + 1) * 512],
                    in0=ps,
                    scalar1=b_sb[:, j : j + 1],
                )

            def v_proj(vT_blk, s):
                ps = work_ps.tile([128, 512], F32, tag="work")
                for e in range(8):
                    nc.tensor.matmul(
                        ps,
                        lhsT=vT_blk[:, e, (s % 4) * 128 : (s % 4 + 1) * 128],
                        rhs=wvT[:, e, :],
                        start=(e == 0),
                        stop=False,
                    )
                nc.tensor.matmul(ps, lhsT=ones_row, rhs=bv_sb, start=False, stop=True)
                nc.vector.tensor_copy(out=vp[:, s, :], in_=ps)

            # ---------------- filler / pending machinery ------------------
            state = {"produced_v": 4, "credit": 0.0, "n_emitted": 0}
            fill_q = deque()  # (rows, fn), single deadline-ordered queue
            pend_q = deque()  # (need_v, fn): PV/normalize closures
            marks = {}

            def drain_pend():
                while pend_q and pend_q[0][0] <= state["produced_v"]:
                    pend_q.popleft()[1]()

            def pump(gain=0.0, flush=False):
                state["credit"] = min(state["credit"] + gain, 5600.0)
                while fill_q and (flush or fill_q[0][0] <= state["credit"]):
                    rows, fn = fill_q.popleft()
                    fn()
                    state["n_emitted"] += 1
                    if not flush:
                        state["credit"] -= rows
                    drain_pend()
                drain_pend()

            def ensure(mark):
                need = marks.get(mark, 0)
                while state["n_emitted"] < need and fill_q:
                    rows, fn = fill_q.popleft()
                    fn()
                    state["n_emitted"] += 1
                    drain_pend()

            def pend_guard(maxlen=8):
                """Bound PV trailing so pt pool slots are never re-allocated
                before their pending reader is emitted (pt bufs=18 > maxlen+1).
                PV may trail up to a whole block; den half ping-pong stays
                safe because norm(g) must drain before block g+2 begins."""
                while len(pend_q) > maxlen:
                    if pend_q[0][0] <= state["produced_v"]:
                        pend_q.popleft()[1]()
                    elif fill_q:
                        rows, fn = fill_q.popleft()
                        fn()
                        state["n_emitted"] += 1
                        drain_pend()
                    else:
                        break

            # ---------------- prologue ------------------------------------
            w_transpose(wkd, wkT, 4, 8)
            w_transpose(wqd, wqT, 4, 8)

            kb0 = load_cast(kd[0:512, :], 512, stage, "xstage")
            for e in range(8):
                transpose_chunk(kT[:, e, 0:512], kb0, e, 4)
            qk_proj(kT, 0, wkT, bk_sb, kp8, 0, 0)

            qb0 = load_cast(qd[0:512, :], 512, stage, "xstage")
            qT0 = xqp.tile([128, 8, 512], FP8, tag="qT")
            for e in range(8):
                transpose_chunk(qT0[:, e, :], qb0, e, 4)
            for jj in range(4):
                qk_proj(qT0, 0, wqT, bq_sb, qp8, jj, 0)

            w_transpose(wvd, wvT, 4, 8)
            vb0 = load_cast(vd[0:512, :], 512, stage, "xstage")
            vT0 = xvp.tile([128, 8, 512], BF16, tag="vT")
            for e in range(8):
                transpose_chunk(vT0[:, e, :], vb0, e, 4)
            for s in range(4):
                v_proj(vT0, s)

            # ---------------- filler units --------------------------------
            staged = {}

            def prefetch(xd, blk, key):
                staged[key] = load_cast(
                    xd[blk * 512 : (blk + 1) * 512, :], 512, stage, "xstage"
                )

            # All remaining input DMAs, queued now in consumption order; the
            # stage pool ring (bufs=4) provides backpressure so ~2 stay in
            # flight ahead of their consumers and none sits on the critical
            # path of a forced staging chain.
            for _xd, _blk, _key in (
                (kd, 1, ("k", 1)),
                (kd, 2, ("k", 2)),
                (kd, 3, ("k", 3)),
                (vd, 1, ("v", 1)),
                (qd, 1, ("q", 1)),
                (vd, 2, ("v", 2)),
                (qd, 2, ("q", 2)),
                (vd, 3, ("v", 3)),
                (qd, 3, ("q", 3)),
            ):
                prefetch(_xd, _blk, _key)

            def mk_xtr1(src_key, dst_fn, e, pop):
                """Transpose ONE e-chunk of a staged x block (512 rows + 1 copy)."""
                def f():
                    xb = staged[src_key]
                    transpose_chunk(dst_fn(e), xb, e, 4)
                    if pop:
                        staged.pop(src_key)

                return f

            def mk_kblk_units(blk):
                dst = lambda e: kT[:, e, blk * 512 : (blk + 1) * 512]
                u = [(512, mk_xtr1(("k", blk), dst, e, e == 7)) for e in range(8)]
                u.append((1024, lambda: qk_proj(kT, blk * 512, wkT, bk_sb, kp8, 0, blk)))
                return u

            def mk_valloc(blk):
                def f():
                    vT = xvp.tile([128, 8, 512], BF16, tag="vT")
                    staged[("vT", blk)] = vT

                return f

            def mk_vblk_units(blk):
                def dst(e):
                    return staged[("vT", blk)][:, e, :]

                u = [(1, mk_valloc(blk))]
                u += [(512, mk_xtr1(("v", blk), dst, e, e == 7)) for e in range(8)]
                return u

            def mk_vproj(blk, s):
                def f():
                    v_proj(staged[("vT", blk)], s)
                    state["produced_v"] = s + 1

                return f

            def mk_qalloc(tb):
                def f():
                    qT = xqp.tile([128, 8, 512], FP8, tag="qT")
                    staged[("qT", tb)] = qT

                return f

            def mk_qT_units(tb):
                def dst(e):
                    return staged[("qT", tb)][:, e, :]

                u = [(1, mk_qalloc(tb))]
                u += [(512, mk_xtr1(("q", tb), dst, e, e == 7)) for e in range(8)]
                return u

            def mk_qproj(j, tb):
                return (1024, lambda: qk_proj(staged[("qT", tb)], 0, wqT, bq_sb, qp8, j, tb))

            def mk_kproj(j, blk):
                return (1024, lambda: qk_proj(kT, blk * 512, wkT, bk_sb, kp8, j, blk))

            def mk_wo_units():
                def fload():
                    staged["wo"] = load_cast(wod, E, wstage, "wstage")

                def mk_tr(e):
                    return lambda: transpose_chunk(woT[:, e, :], staged["wo"], e, 8)

                u = [(1, fload)]
                u += [(1024, mk_tr(e)) for e in range(4)]
                return u

            # single queue, deadline-ordered, v staging interleaved early so
            # PV-pend lag (cap 16) absorbs the production/consumption gap.
            for u in mk_kblk_units(1):
                fill_q.append(u)
            marks[("kblk", 1)] = len(fill_q)
            for u in mk_kblk_units(2):
                fill_q.append(u)
            marks[("kblk", 2)] = len(fill_q)
            for u in mk_vblk_units(1):
                fill_q.append(u)
            fill_q.append((4608, mk_vproj(1, 4)))
            for u in mk_kblk_units(3):
                fill_q.append(u)
            marks[("kblk", 3)] = len(fill_q)
            for s in range(5, 8):
                fill_q.append((4608, mk_vproj(1, s)))
            for blk in range(4):
                fill_q.append(mk_kproj(1, blk))
            marks[(1, 0)] = len(fill_q)
            for u in mk_vblk_units(2):
                fill_q.append(u)
            fill_q.append((4608, mk_vproj(2, 8)))
            fill_q.append((4608, mk_vproj(2, 9)))
            for u in mk_qT_units(1):
                fill_q.append(u)
            for jj in range(4):
                fill_q.append(mk_qproj(jj, 1))
            marks[(0, 1)] = len(fill_q)
            fill_q.append((4608, mk_vproj(2, 10)))
            fill_q.append((4608, mk_vproj(2, 11)))
            for u in mk_vblk_units(3):
                fill_q.append(u)
            fill_q.append((4608, mk_vproj(3, 12)))
            fill_q.append((4608, mk_vproj(3, 13)))
            for u in mk_qT_units(2):
                fill_q.append(u)
            for jj in range(4):
                fill_q.append(mk_qproj(jj, 2))
            marks[(0, 2)] = len(fill_q)
            fill_q.append((4608, mk_vproj(3, 14)))
            fill_q.append((4608, mk_vproj(3, 15)))
            for u in mk_qT_units(3):
                fill_q.append(u)
            for jj in range(4):
                fill_q.append(mk_qproj(jj, 3))
            marks[(0, 3)] = len(fill_q)
            for blk in range(4):
                fill_q.append(mk_kproj(2, blk))
            marks[(2, 3)] = len(fill_q)
            for blk in range(4):
                fill_q.append(mk_kproj(3, blk))
            marks[(3, 3)] = len(fill_q)
            for u in mk_wo_units():
                fill_q.append(u)

            # ---------------- attention -----------------------------------
            den_t = den_ps.tile([128, 16], F32, tag="den")
            denA = den_t[:, 0:8]
            denB = den_t[:, 8:16]

            def attention_block(j, tb, gidx):
                tsl = slice(tb * 512, (tb + 1) * 512)
                den = denA if gidx % 2 == 0 else denB
                # start=True on any matmul wipes co-resident accumulation
                # groups in the same PSUM bank, so zero the region once and
                # accumulate with start=False throughout.
                nc.vector.memset(den, 0.0)
                pts = {}

                def emit_scores(s):
                    ssl = slice(s * 128, (s + 1) * 128)
                    sc = sc_ps.tile([128, 1024], F32, tag="sc")
                    for h in range(2):
                        hp = slice(h * 64, (h + 1) * 64)
                        nc.tensor.matmul(
                            sc[:, h * 512 : (h + 1) * 512],
                            lhsT=kp8[hp, j, ssl].unsqueeze(1).broadcast_to([64, 2, 128]),
                            rhs=qp8[hp, j, tsl].unsqueeze(1).broadcast_to([64, 2, 512]),
                            start=True,
                            stop=True,
                            perf_mode=DR,
                        )
                    pt = ptp.tile([128, 1024], BF16, tag="pt")
                    nc.scalar.activation(
                        out=pt,
                        in_=sc,
                        func=mybir.ActivationFunctionType.Exp,
                        scale=EXP_SCALE,
                    )
                    pts[s] = pt

                def emit_den(s):
                    pt = pts[s]
                    for h in range(2):
                        for tc in range(4):
                            nc.tensor.matmul(
                                den[:, h * 4 + tc : h * 4 + tc + 1],
                                lhsT=pt[:, h * 512 + tc * 128 : h * 512 + tc * 128 + 128],
                                rhs=ones1,
                                start=False,
                                stop=(s == 15),
                                skip_group_check=True,
                            )

                # ctx tile is allocated lazily by the first pv closure so the
                # 1-buf pool rotation lands in pend order.
                box = {}

                def mk_pv(s):
                    def f():
                        if "ctx" not in box:
                            ctx = ctx_ps.tile([128, 512], F32, tag="ctx")
                            box["ctx"] = ctx
                            nc.vector.memset(ctx, 0.0)
                        ctx = box["ctx"]
                        pt = pts.pop(s)
                        for h in range(2):
                            for tc in range(4):
                                nc.tensor.matmul(
                                    ctx[:, (h * 4 + tc) * 64 : (h * 4 + tc) * 64 + 64],
                                    lhsT=pt[:, h * 512 + tc * 128 : h * 512 + tc * 128 + 128],
                                    rhs=vp[:, s, (2 * j + h) * 64 : (2 * j + h + 1) * 64],
                                    start=False,
                                    stop=(s == 15),
                                    skip_group_check=True,
                                )

                    return f

                def mk_norm():
                    def f():
                        ctx = box["ctx"]
                        rec = recp.tile([128, 8], F32, tag="rec")
                        nc.vector.reciprocal(out=rec, in_=den)
                        ctxn = state["ctxn"][tb]
                        for h in range(2):
                            for tc in range(4):
                                nc.vector.tensor_scalar_mul(
                                    out=ctxn[:, tc, (2 * j + h) * 64 : (2 * j + h + 1) * 64],
                                    in0=ctx[:, (h * 4 + tc) * 64 : (h * 4 + tc) * 64 + 64],
                                    scalar1=rec[:, h * 4 + tc : h * 4 + tc + 1],
                                )

                    return f

                emit_scores(0)
                for s in range(16):
                    pend_guard()
                    if s + 1 < 16:
                        if gidx == 0 and (s + 1) % 4 == 0:
                            ensure(("kblk", (s + 1) // 4))
                        emit_scores(s + 1)
                    emit_den(s)
                    pend_q.append((s + 1, mk_pv(s)))
                    pump(1400.0)
                pend_q.append((16, mk_norm()))
                drain_pend()

            def mk_ctxT_outproj(tb):
                """Emitted as filler after (j3, tb): transpose ctx, project out."""
                ctxn = state["ctxn"][tb]
                ctxT = ctxTp.tile([128, 4, 512], BF16, tag="ctxT")

                def mk_tr(ec):
                    def f():
                        wk = work_ps.tile([128, 512], F32, tag="work")
                        tr = wk.bitcast(BF16)
                        for tc in range(4):
                            nc.tensor.transpose(
                                tr[:, tc * 128 : (tc + 1) * 128],
                                ctxn[:, tc, ec * 128 : (ec + 1) * 128],
                                ident,
                            )
                        nc.vector.tensor_copy(out=ctxT[:, ec, :], in_=tr[:, 0:512])

                    return f

                def mk_out(o):
                    def f():
                        ps = work_ps.tile([128, 512], F32, tag="work")
                        for c in range(4):
                            nc.tensor.matmul(
                                ps,
                                lhsT=woT[:, c, o * 128 : (o + 1) * 128],
                                rhs=ctxT[:, c, :],
                                start=(c == 0),
                                stop=(c == 3),
                            )
                        osb = osbp.tile([128, 512], F32, tag="osb")
                        nc.vector.tensor_copy(out=osb, in_=ps)
                        nc.sync.dma_start(
                            out=outd[o * 128 : (o + 1) * 128, tb * 512 : (tb + 1) * 512],
                            in_=osb,
                        )

                    return f

                units = [(512, mk_tr(ec)) for ec in range(4)]
                units += [(2048, mk_out(o)) for o in range(8)]
                return units

            state["ctxn"] = {}
            for tb in range(4):
                ct = ctxnp.tile([128, 4, 512], BF16, tag="ctxn")
                state["ctxn"][tb] = ct

            BLOCKS = [
                (0, 0), (1, 0), (0, 1), (1, 1), (0, 2), (1, 2), (0, 3), (1, 3),
                (2, 3), (3, 3), (2, 0), (3, 0), (2, 1), (3, 1), (2, 2), (3, 2),
            ]
            for gidx, (j, tb) in enumerate(BLOCKS):
                ensure((j, tb))
                attention_block(j, tb, gidx)
                if j == 3:
                    # ctx for this t-block complete: queue its output projection
                    for u in mk_ctxT_outproj(tb):
                        fill_q.append(u)
            pump(flush=True)
            drain_pend()

    legalize_waits(nc)
    return nc


def _make_in_maps(inputs):
    q, k, v = inputs["q"], inputs["k"], inputs["v"]
    in_maps = []
    for c in range(N_CORES):
        b, hh = c // 2, c % 2
        esl = slice(hh * EL, (hh + 1) * EL)
        in_maps.append(
            {
                "q": np.ascontiguousarray(q[b], dtype=np.float32),
                "k": np.ascontiguousarray(k[b], dtype=np.float32),
                "v": np.ascontiguousarray(v[b], dtype=np.float32),
                "wq": np.ascontiguousarray(inputs["Wq"][esl] * WSCALE, dtype=np.float32),
                "wk": np.ascontiguousarray(inputs["Wk"][esl] * WSCALE, dtype=np.float32),
                "wv": np.ascontiguousarray(inputs["Wv"][esl], dtype=np.float32),
                "wo": np.ascontiguousarray(inputs["Wo"][:, esl], dtype=np.float32),
                "bq": np.ascontiguousarray(inputs["bq"][esl] * WSCALE, dtype=np.float32),
                "bk": np.ascontiguousarray(inputs["bk"][esl] * WSCALE, dtype=np.float32),
                "bv": np.ascontiguousarray(inputs["bv"][esl], dtype=np.float32),
            }
        )
    return in_maps


def _gather(results, bo):
    out = np.empty((B, T, E), dtype=np.float32)
    for b in range(B):
        acc = results[2 * b]["outT"].T + results[2 * b + 1]["outT"].T
        out[b] = acc + bo[None, :]
    return out


def run(inputs, **spmd_kwargs):
    if "nc" not in _CACHED:
        _CACHED["nc"] = build_program()
    nc = _CACHED["nc"]
    in_maps = _make_in_maps(inputs)
    res = run_bass_kernel_spmd(nc, in_maps, core_ids=list(range(N_CORES)), **spmd_kwargs)
    out = _gather(res.results, np.asarray(inputs["bo"], dtype=np.float32))
    return out, res


def kernel(**inputs) -> np.ndarray:
    out, _ = run(inputs)
    return out
